# revision 52
# baseline (speedup 1.0000x reference)
"""Trainium2 Bass kernel for the pairwise-MLP geometric convolution.

Reference computes, per batch z:
    rel[a,b]   = g[b] - g[a]
    h[a,b,:]   = relu(rel @ W1 + b1)                      [N,N,H]
    k[a,b,:]   = h @ W2 + b2  -> [N,N,C_OUT,C_IN]
    out[a,i]   = sum_{b,j} k[a,b,i,j] * f[b,j]

Key factorization (avoids materializing k):
    Ua  = g @ W1                (rel@W1 + b1 = Ub' - Ua, b1 folded into Ub')
    Ub' = [g_q, 1] @ [W1; b1]
    G[b,h,i] = sum_j W2[h, i*C_IN+j] * f[b,j]
    out[a,i] = sum_{b,h} relu(Ub'[b,h] - Ua[a,h]) * G[b,h,i]
             + sum_j b2[i,j] * (sum_b f[b,j])

Sharding over 8 cores: z (2) x b-quarter (4). Each core computes the full
[a=256, i=32] partial for its 64 b's; host sums quarters.

Layout trick that avoids any DRAM-bounce regroup of G: G is computed
TRANSPOSED, one matmul per output channel i, with the W2 slice as the
stationary operand:
    g_ps[h, i*64+b] = sum_j m2p2[j, i*64+h] * fTq[j, b]
so h lands on partitions directly. Strided partition-window copies then build
g3[(hl,h), p, i] = G[2p+hl, h, i]  (hl = b parity) in SBUF, and the main
contraction runs as 64 accumulating matmuls with K = (hl,h) = 128:
    acc[a_half, i] += t_p[:, a_half]^T @ g3[:, p, :]
where t_p[(hl,h), a] = relu(Ub'[2p+hl,h] - Ua[a,h]) is one tensor_scalar per
b-pair (bf16 in/out -> 4x DVE mode). The b2 bias is accumulated into its own
PSUM tile off the critical path and added during the output copy.

Hardware constraint honored throughout: a TPB instruction can carry at most
ONE sync-wait (walrus codegen limit). Shared T inputs are placed so every
consumer needs one wait (ub2 on DVE, negua2 halves on DVE+ACT, a Pool fence
op), and dummy PE matmuls observe the g3 copy semaphores before the main
accumulation chain.
"""

import os
import sys

import numpy as np

_TRN_REPO = "/opt/trn_rl_repo"
if _TRN_REPO not in sys.path:
    sys.path.insert(0, _TRN_REPO)

from contextlib import ExitStack

import concourse.bass as bass
import concourse.mybir as mybir
import concourse.tile as tile
from concourse.bass_utils import run_bass_kernel_spmd

from concourse.vector_clock import ScopedClock

# The walrus codegen used on the axon/PJRT path accepts at most ONE sync-wait
# per TPB instruction. Tile's kernel-tail drain aggregates a wait for every
# live semaphore onto a single Drain, which walrus rejects. Patch the tail to
# spread those waits across single-wait SP nops before an unadorned drain.
_orig_drain_and_barrier = tile.TileContext._drain_and_barrier

# manual semaphores to clear at kernel tail for warm-run correctness
_MANUAL_SEMS: list = []


def _split_wait_drain_and_barrier(self, tick_clock, wait_clock):
    nc = self.nc
    probe = nc.sync.nop(nofuse=True)
    wait_clock.add_sem_waits(probe.ins, ScopedClock({None: tick_clock.global_clock}))
    si = probe.ins.sync_info
    waits = list(si.on_wait) if si is not None and si.on_wait else []
    if len(waits) > 1:
        probe.ins.sync_info = mybir.SyncInfo(on_wait=waits[:1], on_update=[])
        for w in waits[1:]:
            extra = nc.sync.nop(nofuse=True)
            extra.ins.sync_info = mybir.SyncInfo(on_wait=[w], on_update=[])
    nc.sync.drain()
    nc.all_engine_barrier()
    popped = nc._tile_sem_poison_stack.pop()
    assert popped is self._sem_poison
    sems = list(self.sems.allocated().values())
    sems.extend(_MANUAL_SEMS)
    _MANUAL_SEMS.clear()
    nc.clear_and_free_semaphores(sems)
    # no trailing all-engine barrier: per-engine queues are FIFO across NEFF
    # executions, so run N+1's init barrier cannot pass until Pool finishes
    # this clear; the extra ~200ns barrier adds nothing.


tile.TileContext._drain_and_barrier = _split_wait_drain_and_barrier

F32 = mybir.dt.float32
BF16 = mybir.dt.bfloat16
Z, N, C_IN, C_OUT, H = 2, 256, 32, 32, 64
BQ = 64          # b-points per core (N / 4 quarters)
NPAIR = BQ // 2  # 32 K-chunks of (2 b x 64 h) = 128

# packed bf16 tensor (matmul operands) [32, MPW]:
#   cols 0:256      gT      (parts 0:3)   g[z].T for Ua
#   cols 256:288    gq2     (parts 0:8)   gq2[hl*4+x, p] = g[z,q*64+2p+hl,x]; x=3 -> 1
#   cols 320:448    w1d     (parts 0:3)   W1 duplicated: w1d[x, hl*64+h] = W1[x,h]
#   cols 448:576    ones    (part 0)      lhsT for the bias rank-1 matmuls
#   cols 576:640    fTq     (parts 0:32)  f[z,quarter].T
#   cols 640:768    w1blk   (parts 0:8)   block-diag [W1;b1] over (hl,h)
#   cols 768:1280   m2p2[i<8]  (0:32)    G bank-0 weights ride DMA-1 so the
#                                         serial ACT regroup chain (the
#                                         kernel gate) starts ~700ns earlier
#   cols 1280:1344  spare
#   cols 1344:2880  m2p2[i>=8] (0:32)    G bank 1-3 weights (DMA-2)
#   cols 2816..     (b2t2 moved to 576+: see B2T2 below)
MPW = 2880
D1A = 1344  # first DMA: U path + fTq + bank-0 m2p2
B2T2 = 1280

# engine for each of the 32 T-chunk builds: v=vector(DVE), g=gpsimd.
# ACT is saturated by the 8 g3 doubling copies.
T_ENGINES = [
    "v", "v", "v", "v", "g", "v", "v", "v",
    "v", "g", "v", "v", "v", "v", "g", "v",
    "v", "v", "v", "g", "v", "v", "v", "g",
    "v", "v", "v", "g", "v", "g", "v", "v",
]


def build_nc(debug: bool = False) -> bass.Bass:
    nc = bass.Bass("TRN2", target_bir_lowering=False, debug=debug, num_devices=8)

    mp = nc.dram_tensor("mp", [C_IN, MPW], BF16, kind="ExternalInput").ap()
    outp = nc.dram_tensor("outp", [N, C_OUT], F32, kind="ExternalOutput").ap()

    # pre-TileContext input DMAs on manual sems, hoisted before the
    # Bass-init barrier so the DMA latency overlaps the prologue.
    mp_sb = nc.alloc_sbuf_tensor("mp_sb", [C_IN, MPW], BF16).ap()
    in_a = nc.alloc_semaphore("in_a")
    in_b = nc.alloc_semaphore("in_b")
    _MANUAL_SEMS.extend([in_a, in_b])
    hoist = [
        nc.sync.dma_start(out=mp_sb[:, 0:D1A],
                          in_=mp[:, 0:D1A]).then_inc(in_a, 16),
        nc.sync.dma_start(out=mp_sb[:, D1A:MPW],
                          in_=mp[:, D1A:MPW]).then_inc(in_b, 16),
    ]

    from concourse.tile_rust import add_dep_helper

    deferred_waits: list = []

    def dep(after, before):
        add_dep_helper(after.ins, before.ins, sync=False, reason="manual order")

    def fence(engine, sem, val):
        probe = engine.nop(nofuse=True)
        deferred_waits.append((probe, sem, val))
        return probe

    with tile.TileContext(nc) as tc, ExitStack() as ctx:
        work = ctx.enter_context(tc.tile_pool(name="work", bufs=1))
        # bufs=NPAIR: every T tile gets its own slot, so no T-op ever waits
        # for a PE slot release (keeps every instruction at <=1 sync wait).
        tpool = ctx.enter_context(tc.tile_pool(name="tpool", bufs=NPAIR))
        psum = ctx.enter_context(tc.tile_pool(name="psum", bufs=1, space="PSUM"))

        f_pe_a = fence(nc.tensor, in_a, 16)
        f_dve_a = fence(nc.vector, in_a, 16)

        gT = mp_sb[0:3, 0:256]
        gq2 = mp_sb[0:8, 256:288]
        w1d = mp_sb[0:3, 320:448]
        w1blk = mp_sb[0:8, 640:768]
        ones_r = mp_sb[0:1, 448:576]
        fTq = mp_sb[:, 576:640]
        b2t_bf = mp_sb[:, B2T2:B2T2 + 64]

        # ---- U matmuls, directly in (hl,h)-partition layout: duplicated W1
        # gives +Ua on both partition halves in one matmul; block-diagonal
        # [W1;b1] with parity-split gq gives ub2[(hl,h), p] = Ub'[2p+hl, h].
        u_ps = psum.tile([2 * H, 288], F32)
        ua2_ps = u_ps[:, 32:288]
        ub2_ps = u_ps[:, 0:32]
        umm1 = nc.tensor.matmul(ua2_ps, lhsT=w1d, rhs=gT, start=True, stop=True)
        umm2 = nc.tensor.matmul(ub2_ps, lhsT=w1blk, rhs=gq2,
                                start=True, stop=True)
        dep(umm1, f_pe_a)
        dep(umm2, f_pe_a)

        # scol[j] = sum_{b in quarter} f[z,b,j] (host unshard completes the
        # b sum). First in the DVE queue: it only needs the first DMA and
        # runs inside the window where ub2 would wait on the U matmuls.
        scol = work.tile([C_IN, 1], BF16)
        with nc.allow_low_precision(reason="bias rank-1 term, tolerance 2e-2"):
            scol_op = nc.vector.tensor_reduce(out=scol, in_=fTq,
                                              axis=mybir.AxisListType.X,
                                              op=mybir.AluOpType.add)
        dep(scol_op, f_dve_a)

        # ALL shared T-op inputs live on DVE: the tile scheduler emits one
        # watermark wait per distinct producer engine, so a consumer on any
        # engine then needs exactly one (DVE) wait.
        ub2 = work.tile([2 * H, NPAIR], F32)
        nc.vector.tensor_copy(ub2, ub2_ps)
        negua2 = work.tile([2 * H, N], BF16)
        nc.vector.tensor_scalar(out=negua2, in0=ua2_ps,
                                scalar1=-1.0, scalar2=None,
                                op0=mybir.AluOpType.mult)
        # ---- G matmuls: g_ps[h, i*64+b] = sum_j m2p2[j, i*64+h] fTq[j, b].
        # One matmul per i; the W2 slice is the stationary operand so h lands
        # on partitions. Four 1-bank PSUM tiles, 8 i-slices each.
        gb = []
        for k in range(4):
            gp = psum.tile([BQ, 512], F32, name=f"g_ps{k}", tag=f"g_ps{k}")
            gb.append(gp)
        misc_w = 2 * C_OUT + 1
        f_pe_b = fence(nc.tensor, in_b, 16)
        dep(f_pe_b, umm2)
        dep(f_pe_b, f_pe_a)
        for i in range(C_OUT):
            k, off = divmod(i, 8)
            base = 768 + i * 64 if i < 8 else 1344 + (i - 8) * 64
            gmm = nc.tensor.matmul(gb[k][:, off * 64:(off + 1) * 64],
                                   lhsT=mp_sb[:, base:base + 64],
                                   rhs=fTq, start=True, stop=True)
            dep(gmm, f_pe_a if i < 8 else f_pe_b)

        # ---- T tiles: t_p[(hl,h), a] = relu(Ub'[2p+hl,h] - Ua[a,h]).
        t_tiles = []
        for p in range(NPAIR):
            t_p = tpool.tile([2 * H, N], BF16, tag="T", name=f"t_{p}")
            t_tiles.append(t_p)
            eng = T_ENGINES[p]
            if eng == "s":
                nc.scalar.activation(t_p, negua2,
                                     mybir.ActivationFunctionType.Relu,
                                     bias=ub2[:, p:p + 1], scale=1.0)
            else:
                e = nc.vector if eng == "v" else nc.gpsimd
                e.tensor_scalar(out=t_p, in0=negua2,
                                scalar1=ub2[:, p:p + 1], scalar2=0.0,
                                op0=mybir.AluOpType.add,
                                op1=mybir.AluOpType.max)

        # ---- partition-doubling copies on ACT: g3[(hl,h), p, i] =
        # G[2p+hl, h, i] = g_ps[h, i*64 + 2p+hl]. One strided copy per
        # (bank, hl) so each op carries a single PE-semaphore wait and starts
        # as soon as its bank's G matmuls are done.
        g3 = work.tile([2 * H, NPAIR, C_OUT], BF16)
        misc_ps = psum.tile([1, misc_w], F32)
        b2s_ps = misc_ps[:, 0:2 * C_OUT]
        scrap = misc_ps[:, 2 * C_OUT:misc_w]
        b2s_sb = work.tile([1, 2 * C_OUT], BF16)

        def dbl(k, hl, eng="a"):
            srcv = gb[k].rearrange("h (i p two) -> h two p i", two=2, p=NPAIR)
            dst = g3[hl * H:(hl + 1) * H, :, 8 * k:8 * (k + 1)]
            if eng == "a":
                nc.scalar.activation(dst, srcv[:, hl, :, :],
                                     mybir.ActivationFunctionType.Copy)
            else:
                nc.vector.tensor_scalar(out=dst, in0=srcv[:, hl, :, :],
                                        scalar1=1.0, scalar2=None,
                                        op0=mybir.AluOpType.mult)

        dbl(0, 0)
        dbl(0, 1)
        # b2s[i] = sum_j b2t[j,i] scol[j] on PE right after the G matmuls
        # (which already observed the D1b DMA); the SBUF copy slots into the
        # ACT queue between g3 copies. By the time the bias enders run after
        # the mains, everything is long satisfied.
        nc.tensor.matmul(b2s_ps, lhsT=scol, rhs=b2t_bf, start=True, stop=True)
        nc.scalar.activation(b2s_sb, b2s_ps, mybir.ActivationFunctionType.Copy)
        for k in range(1, 4):
            for hl in range(2):
                dbl(k, hl)

        # PE observes the 8 g3 copy semaphores (one wait each) so the main
        # matmuls need only their T-tile wait.
        for k in range(4):
            for hl in range(2):
                nc.tensor.matmul(scrap,
                                 lhsT=g3[hl * H:(hl + 1) * H, 0, 8 * k:8 * k + 1],
                                 rhs=g3[hl * H:(hl + 1) * H, 0, 8 * k:8 * k + 1],
                                 start=True, stop=True)

        # ---- main contraction into ONE [128, 64] acc bank. A full-width
        # start=True bias STARTER (ones^T @ (b2s|b2s)) zeroes the bank and
        # deposits the bias over every byte first, so all 64 mains are
        # plain start=False accumulates onto initialized PSUM (one group,
        # no enders) and the tail needs a single PSUM->SBUF copy.
        acc = psum.tile([2 * H, 2 * C_OUT], F32)
        nc.tensor.matmul(acc, lhsT=ones_r, rhs=b2s_sb, start=True, stop=False)
        for p in range(NPAIR):
            for half in range(2):
                nc.tensor.matmul(acc[:, half * C_OUT:(half + 1) * C_OUT],
                                 lhsT=t_tiles[p][:, half * 2 * H:(half + 1) * 2 * H],
                                 rhs=g3[:, p, :],
                                 start=False,
                                 stop=(p == NPAIR - 1 and half == 1))

        # ---- store: out[a, i], a = half*128 + ap.
        out_sb = work.tile([2 * H, 2 * C_OUT], F32)
        nc.vector.tensor_copy(out_sb, acc)
        srcv = bass.AP(tensor=out_sb.tensor, offset=out_sb.offset,
                       ap=[[2 * C_OUT, 2 * H], [C_OUT, 2], [1, C_OUT]])
        dstv = bass.AP(tensor=outp.tensor, offset=outp.offset,
                       ap=[[C_OUT, 2 * H], [2 * H * C_OUT, 2], [1, C_OUT]])
        nc.sync.dma_start(out=dstv, in_=srcv)

    for probe, sem, val in deferred_waits:
        probe._wait_ge(sem, val)

    # hoist the input DMAs before the Bass-init barrier/const-memsets
    blk = nc.m.functions[0].blocks[0]
    names = {h.ins.name for h in hoist}
    moved = [i for i in blk.instructions if i.name in names]
    rest = [i for i in blk.instructions if i.name not in names]
    cut = 0
    for kk, i in enumerate(rest):
        if i.opcode in ("Memset", "Drain"):
            cut = kk
            break
    blk.instructions = rest[:cut] + moved + rest[cut:]

    return nc

def shard_inputs(features, geometry, W1, b1, W2, b2) -> list[dict]:
    import ml_dtypes
    bf16 = ml_dtypes.bfloat16
    f = np.ascontiguousarray(np.asarray(features, np.float32))
    g = np.ascontiguousarray(np.asarray(geometry, np.float32))
    W1 = np.ascontiguousarray(np.asarray(W1, np.float32))
    b1 = np.ascontiguousarray(np.asarray(b1, np.float32))
    W2 = np.ascontiguousarray(np.asarray(W2, np.float32))
    b2 = np.ascontiguousarray(np.asarray(b2, np.float32))

    # m2p2[j, i*64+h] = W2[h, i*C_IN+j]
    m2p2 = W2.reshape(H, C_OUT, C_IN).transpose(2, 1, 0).reshape(C_IN, C_OUT * H)
    b2t = np.ascontiguousarray(b2.reshape(C_OUT, C_IN).T)

    maps = []
    for core in range(8):
        z, q = divmod(core, 4)
        sl = slice(q * BQ, (q + 1) * BQ)
        mp = np.zeros((C_IN, MPW), bf16)
        mp[0:3, 0:256] = g[z].T.astype(bf16)
        gq = g[z, sl]  # [BQ, 3]
        for hl in range(2):
            mp[hl * 4:hl * 4 + 3, 256:288] = gq[hl::2].T.astype(bf16)
            mp[hl * 4 + 3, 256:288] = 1.0
            mp[0:3, 320 + hl * 64:320 + (hl + 1) * 64] = W1.astype(bf16)
            mp[hl * 4:hl * 4 + 3, 640 + hl * 64:640 + (hl + 1) * 64] = \
                W1.astype(bf16)
            mp[hl * 4 + 3, 640 + hl * 64:640 + (hl + 1) * 64] = b1.astype(bf16)
        mp[0, 448:576] = 1.0
        mp[:, 576:640] = f[z, sl].T.astype(bf16)
        mp[:, 768:1280] = m2p2[:, 0:512].astype(bf16)
        mp[:, 1280:1312] = b2t.astype(bf16)
        mp[:, 1312:1344] = b2t.astype(bf16)
        mp[:, 1344:2880] = m2p2[:, 512:2048].astype(bf16)
        maps.append({"mp": mp})
    return maps


def unshard(parts: list[np.ndarray]) -> np.ndarray:
    out = np.empty((Z, N, C_OUT), np.float32)
    for z in range(Z):
        acc = parts[4 * z].astype(np.float32)
        for q in range(1, 4):
            acc = acc + parts[4 * z + q]
        out[z] = acc
    return out


def kernel(**inputs) -> np.ndarray:
    nc = build_nc(debug=False)
    in_maps = shard_inputs(**inputs)
    res = run_bass_kernel_spmd(nc, in_maps, list(range(8)))
    return unshard([r["outp"] for r in res.results])

